# revision 7
# baseline (speedup 1.0000x reference)
"""Pairwise-interaction kernel for Trainium2 (raw Bass), 8-core SPMD.

Computes out[b, p, :] = x[b, i(p), :] * x[b, j(p), :] for all pairs
(i < j) of the F=26 feature rows, p ordered row-major (i outer, j inner).

Sharding: data-parallel over the batch dim (16384 -> 8 x 2048), no
cross-core communication.

v6 design notes:
  * All tensors bf16: DVE 2x packing mode doubles tensor_tensor
    throughput vs f32 AND halves HBM traffic. The added rounding error
    (~1.2% worst case) is well inside the 2e-2 gate; f32<->bf16
    conversion happens on the host.
  * Samples are interleaved G=4 per partition row (sample =
    t*P*G + p*G + g): every TT instruction covers all 4 groups
    (amortizes the ~58-cycle DVE bubble; DVE ~95us total) and DMA
    descriptor rows are multi-KB contiguous DRAM runs.
  * The exec floor is the store stream: first-chunk-ready +
    42.6MB / ~425GB/s (16 SDMA engines shared by both HWDGE rings, so
    all stores ride the sync ring; loads ride the scalar ring so they
    never queue behind stores). Chunks are pair-ranges sized
    tiny/huge/small so the stream starts ~2us into the first sweep and
    the post-compute tail is ~2.8us.
  * Supertile 0 is special-cased into two half-group (GS=2) loads and
    sweeps so the first store chunk is ready as early as possible.

Raw-Bass sync scheme (one semaphore wait per instruction; extra
ordering uses standalone wait_ge ops on the engine queue):
  sem_ld (+16 per load DMA, scalar ring; 2 half-loads for t=0)
  sem_st (+16 per store DMA, sync ring)
  sem_tt (+1 by the last TT of each chunk, vector engine)
"""

import numpy as np
import ml_dtypes

import concourse.bass as bass
from concourse import mybir
from concourse.bass_utils import run_bass_kernel_spmd

B, F, D = 16384, 26, 32
NCORES = 8
BC = B // NCORES           # 2048 samples per core
P = 128                    # SBUF partitions
G = 4                      # sample groups per supertile (consecutive rows)
GS = G // 2                # groups per half-sweep (supertile 0 only)
NTS = BC // (P * G)        # 4 supertiles per core
FD = F * D                 # 832
NPAIR = F * (F - 1) // 2   # 325
OD = NPAIR * D             # 10400

XB = NTS                   # all input supertiles resident at once
YB = 2                     # output supertile buffers

# i-block ranges per store chunk: pair counts (25, 264, 36) - tiny
# first chunk so the store stream starts early, one huge middle chunk
# (16.9KB descriptor rows sustain peak DMA rate), small last chunk so
# the post-compute drain is short.
CHUNKS = [(0, 1), (1, 17), (17, 25)]
NCH = len(CHUNKS)

BF16 = mybir.dt.bfloat16
NP_BF16 = ml_dtypes.bfloat16


def _pair_off(i_lo):
    return sum(F - 1 - i for i in range(i_lo))


# store ordinal (1-based) of chunk c of supertile t on the sync ring:
# t=0 is stored as two half-group passes of NCH chunks each
def _st_ord(t, c):
    return 2 * NCH + NCH * (t - 1) + c + 1 if t >= 1 else NCH + c + 1


_nc_cache = None


def _build_nc():
    nc = bass.Bass()
    x = nc.declare_dram_parameter("x", [BC, FD], BF16, isOutput=False)
    y = nc.declare_dram_parameter("y", [BC, OD], BF16, isOutput=True)
    # sample s = t*P*G + p*G + g: partition p's G samples are consecutive
    # DRAM rows, so per-partition DMA runs are long and contiguous.
    xv = x[:].rearrange("(t p g) m -> t p (g m)", p=P, g=G)
    yv = y[:].rearrange("(t p g) m -> t p g m", p=P, g=G)

    with (
        nc.sbuf_tensor([P, XB * G * FD], BF16) as xbuf,
        nc.sbuf_tensor([P, YB * G * OD], BF16) as ybuf,
        nc.semaphore("sem_ld") as sem_ld,
        nc.semaphore("sem_st") as sem_st,
        nc.semaphore("sem_tt") as sem_tt,
        nc.Block() as blk,
    ):
        xts = [xbuf[:, b * G * FD : (b + 1) * G * FD] for b in range(XB)]
        yts = [ybuf[:, b * G * OD : (b + 1) * G * OD] for b in range(YB)]

        def sweep(v, xt, yt, g_lo, g_hi, tt_base):
            # TT sweep over groups [g_lo, g_hi), one sem_tt inc per chunk
            ng = g_hi - g_lo
            for i_lo, i_hi in CHUNKS:
                off = _pair_off(i_lo)
                for i in range(i_lo, i_hi):
                    nrep = F - 1 - i
                    in0 = (
                        xt[:, g_lo:g_hi, i * D : (i + 1) * D]
                        .unsqueeze(2)
                        .broadcast_to([P, ng, nrep, D])
                    )
                    in1 = xt[:, g_lo:g_hi, (i + 1) * D : FD].rearrange(
                        "p g (r d) -> p g r d", d=D
                    )
                    outap = yt[
                        :, g_lo:g_hi, off * D : (off + nrep) * D
                    ].rearrange("p g (r d) -> p g r d", d=D)
                    tt = nc.vector.tensor_mul(outap, in0, in1)
                    off += nrep
                tt.then_inc(sem_tt, 1)

        @blk.scalar
        def _(scalar):
            # supertile 0 loads in two half-group pieces so the first
            # sweep starts as early as possible
            for h in range(2):
                scalar.dma_start(
                    xts[0][:, h * GS * FD : (h + 1) * GS * FD],
                    xv[0][:, h * GS * FD : (h + 1) * GS * FD],
                ).then_inc(sem_ld, 16)
            for t in range(1, NTS):
                scalar.dma_start(xts[t], xv[t]).then_inc(sem_ld, 16)

        @blk.sync
        def _(sync):
            yt0 = yts[0].rearrange("p (g m) -> p g m", g=G)
            for h in range(2):
                for c, (i_lo, i_hi) in enumerate(CHUNKS):
                    p_lo, p_hi = _pair_off(i_lo), _pair_off(i_hi)
                    st = sync.dma_start(
                        yv[0][:, h * GS : (h + 1) * GS, p_lo * D : p_hi * D],
                        yt0[:, h * GS : (h + 1) * GS, p_lo * D : p_hi * D],
                    )
                    st._wait_ge(sem_tt, NCH * h + c + 1)
                    st.then_inc(sem_st, 16)
            for t in range(1, NTS):
                yt = yts[t % YB].rearrange("p (g m) -> p g m", g=G)
                for c, (i_lo, i_hi) in enumerate(CHUNKS):
                    p_lo, p_hi = _pair_off(i_lo), _pair_off(i_hi)
                    st = sync.dma_start(
                        yv[t][:, :, p_lo * D : p_hi * D],
                        yt[:, :, p_lo * D : p_hi * D],
                    )
                    st._wait_ge(sem_tt, NCH * (t + 1) + c + 1)
                    st.then_inc(sem_st, 16)

        @blk.vector
        def _(v):
            # supertile 0: two half-group sweeps, gated on the half-loads
            xt0 = xts[0].rearrange("p (g m) -> p g m", g=G)
            yt0 = yts[0].rearrange("p (g m) -> p g m", g=G)
            for h in range(2):
                v.wait_ge(sem_ld, 16 * (h + 1))
                sweep(v, xt0, yt0, h * GS, (h + 1) * GS, NCH * h)
            for t in range(1, NTS):
                xt = xts[t].rearrange("p (g m) -> p g m", g=G)
                yt = yts[t % YB].rearrange("p (g m) -> p g m", g=G)
                v.wait_ge(sem_ld, 16 * (t + 2))
                for c, (i_lo, i_hi) in enumerate(CHUNKS):
                    if t >= YB:
                        # chunk c of ybuf slot t-YB fully stored (for the
                        # t=0 tenant that means both half-group passes)
                        v.wait_ge(sem_st, 16 * _st_ord(t - YB, c))
                    off = _pair_off(i_lo)
                    for i in range(i_lo, i_hi):
                        nrep = F - 1 - i
                        in0 = (
                            xt[:, :, i * D : (i + 1) * D]
                            .unsqueeze(2)
                            .broadcast_to([P, G, nrep, D])
                        )
                        in1 = xt[:, :, (i + 1) * D : FD].rearrange(
                            "p g (r d) -> p g r d", d=D
                        )
                        outap = yt[
                            :, :, off * D : (off + nrep) * D
                        ].rearrange("p g (r d) -> p g r d", d=D)
                        tt = nc.vector.tensor_mul(outap, in0, in1)
                        off += nrep
                    tt.then_inc(sem_tt, 1)

    return nc


def _make_in_maps(inputs: np.ndarray):
    x = np.asarray(inputs, dtype=np.float32).reshape(B, FD).astype(NP_BF16)
    shards = np.ascontiguousarray(x.reshape(NCORES, BC, FD))
    return [{"x": shards[c]} for c in range(NCORES)]


def kernel(inputs: np.ndarray) -> np.ndarray:
    global _nc_cache
    if _nc_cache is None:
        _nc_cache = _build_nc()
    nc = _nc_cache

    in_maps = _make_in_maps(inputs)
    res = run_bass_kernel_spmd(nc, in_maps, list(range(NCORES)))
    out = np.concatenate([res.results[c]["y"] for c in range(NCORES)], axis=0)
    return out.astype(np.float32).reshape(B, NPAIR, D)
